# revision 1
# baseline (speedup 1.0000x reference)
"""Trainium2 Bass kernel for nn_Decoder_67705864454693.

Module: 4-head LinearOutputStack MLP (loc/var/freq/amp per event) ->
sum_e amp*sin(freq*pi*n)*NormalPDF(loc,var)(rng[n]) over n=1..32768 -> max-norm.

Sharding: data-parallel over batch B=8, one batch per NeuronCore (8 cores).

Per-core device algorithm (partition dim = 128 events, free dim = sample
tiles of 512):
  1. MLP in fp32 matmuls (freq must match the reference's f32 value to ~ulp
     since phase errors amplify by pi*n ~ 1e5); Lrelu on ACT fuses bias.
  2. Head scalars via exp-route sigmoid (ACT exp table is ~2ulp) + DVE
     reciprocal; all per-event coefficients built on device and bf16-split
     so the hot-loop matmuls are exact.
  3. Phase A (all 64 tiles): sin argument m = s*(k+1) (s ~= fl(freq*pi)/2pi
     in 3 bf16 pieces) is range-reduced ON THE TENSOR ENGINE with the
     magic-constant trick: mm1 accumulates -(m) then -C (C=1.5*2^23) giving
     -(C+round(m)) under RNE; mm2 accumulates +C then +m giving
     u = m-round(m) in [-0.5,0.5]. ACT Sin(scale=2pi) -> bf16 sin buffer.
     (Sin domain is [-pi,pi]; HW clamps the rare half-ulp overshoot.)
  4. Phase B (all 64 tiles): z^2 = a^2*k^2+2ab*k+b^2 as a K=12 split-bf16
     matmul; ACT Exp(-z^2/2) -> bf16; DVE scalar_tensor_tensor
     (bump*c[e])*sin -> prod; PE matmul with a ones-band lhsT accumulates
     the event sum for tile t into PSUM row t -> out laid out [64, 512].
  5. abs-max over free dim + tiny transpose-by-DMA for the partition max,
     reciprocal, scale, DMA out.

Two-phase structure = one Sin->Exp ACT table switch total instead of 128.
"""
import numpy as np
import ml_dtypes

bfnp = ml_dtypes.bfloat16

N = 32768
E = 128
D = 128
NT = 512
T = N // NT            # 64 tiles
C_MAGIC = 12582912.0   # 1.5 * 2^23
NB = 8                 # batches / cores

_cached = {}


def _make_basis():
    """Host-precomputed bf16 basis rows (exact splits).

    basisA1: mm1 moving rows; basisA2: mm2 moving rows (the magic-C row must
    accumulate LAST in mm1 and FIRST in mm2 — row order set by
    ORDER_LOW_FIRST). basisB: z^2 rows."""
    k = np.arange(N, dtype=np.int64)
    k1 = k + 1
    k1_hi = (k1 // 256).astype(np.float32)   # 0..128, bf16-exact
    k1_lo = (k1 % 256).astype(np.float32)    # 0..255, bf16-exact
    j_hi = (k // 256).astype(np.float32)
    j_lo = (k % 256).astype(np.float32)
    ones = np.ones(N, np.float32)
    j2 = (k.astype(np.float64)) ** 2
    j2a = j2.astype(bfnp)
    r = j2 - j2a.astype(np.float64)
    j2b = r.astype(bfnp)
    r2 = r - j2b.astype(np.float64)
    j2c = r2.astype(bfnp)

    # The PE's in-matmul K-accumulation is a pairwise tree (HW-probed), so
    # the magic-C row cannot ride inside a K=7 matmul. Instead the sin-arg
    # uses 4 PSUM-accumulating matmuls (across-matmul accumulation is exact
    # per-element RMW): tree(-m) [K=6] -> -C [K=1] -> +C [K=1] -> +tree(m).
    basisM = np.stack([k1_hi, k1_hi, k1_hi, k1_lo, k1_lo, k1_lo]
                      ).astype(bfnp)      # [6, N]

    # basisB rows: [j2a,j2a, j2b,j2b, j2c,j2c, j_hi,j_hi, j_lo,j_lo, 1,1]
    basisB = np.stack([
        j2a, j2a, j2b, j2b, j2c, j2c,
        j_hi.astype(bfnp), j_hi.astype(bfnp),
        j_lo.astype(bfnp), j_lo.astype(bfnp),
        ones.astype(bfnp), ones.astype(bfnp),
    ])  # [12, N]

    # ones-band for the event-sum output placement: col 63 is all-ones;
    # lhsT slice [:, 63-t:127-t] has its ones in column t.
    eband = np.zeros((128, 127), np.float32)
    eband[:, 63] = 1.0
    return basisM, basisB.astype(bfnp), eband.astype(bfnp)


def _build(debug=False):
    from contextlib import ExitStack
    import concourse.bass as bass
    import concourse.tile as tile
    from concourse import mybir
    from concourse.bass import ts

    F32 = mybir.dt.float32
    BF16 = mybir.dt.bfloat16
    A = mybir.ActivationFunctionType
    OP = mybir.AluOpType

    nc = bass.Bass()

    # ---- I/O ----
    xT_d = nc.dram_tensor("xT", [D, E], F32, kind="ExternalInput")
    wsT_d = nc.dram_tensor("wsT", [12, D, D], F32, kind="ExternalInput")
    bsT_d = nc.dram_tensor("bsT", [12, D], F32, kind="ExternalInput")
    woT_d = nc.dram_tensor("woT", [4, D], F32, kind="ExternalInput")
    bor_d = nc.dram_tensor("bor", [1, 4 * D], F32, kind="ExternalInput")
    out_d = nc.dram_tensor("out", [T, NT], F32, kind="ExternalOutput")

    basisM_np, basisB_np, eband_np = _make_basis()
    basisM_d = nc.inline_tensor(basisM_np, name="basisM")     # [6, N] bf16
    basisB_d = nc.inline_tensor(basisB_np, name="basisB")     # [12, N] bf16
    eband_d = nc.inline_tensor(eband_np, name="eband")        # [128, 127]

    scrA_d = nc.dram_tensor("scrA", [1, 12 * D], BF16, kind="Internal")
    scrB_d = nc.dram_tensor("scrB", [1, 12 * D], BF16, kind="Internal")
    scrC_d = nc.dram_tensor("scrC", [1, D], F32, kind="Internal")
    scrM_d = nc.dram_tensor("scrM", [T, 1], F32, kind="Internal")
    scrI_d = nc.dram_tensor("scrI", [1, 1], F32, kind="Internal")

    dbg = {}
    if debug:
        for nm, shape, dt in [
            ("dbg_scores", [1, 4 * D], F32), ("dbg_sig", [1, 4 * D], F32),
            ("dbg_cst", [1, 16 * D], F32), ("dbg_h0", [D, E], F32),
            ("dbg_w0", [D, D], F32),
            ("dbg_lhsA1", [6, D], BF16), ("dbg_lhsA2", [6, D], BF16),
            ("dbg_lhsB", [12, D], BF16), ("dbg_cwcol", [D, 1], F32),
            ("dbg_pm", [E, NT], F32), ("dbg_sin", [E, NT], BF16),
            ("dbg_z2", [E, NT], F32), ("dbg_bump", [E, NT], BF16),
            ("dbg_prod", [E, NT], BF16), ("dbg_outsb", [T, NT], F32),
        ]:
            dbg[nm] = nc.dram_tensor(nm, shape, dt, kind="ExternalOutput")

    TWO_PI = float(2.0 * np.pi)
    INV_2PI_HI = float(np.float32(1.0 / (2.0 * np.pi)))
    INV_2PI_LO = float(np.float32(
        1.0 / (2.0 * np.pi) - np.float64(np.float32(1.0 / (2.0 * np.pi)))))
    INV_NM1 = float(np.float32(1.0 / (N - 1)))
    INV_SQRT_2PI = float(np.float32(1.0 / np.sqrt(2.0 * np.pi)))

    with tile.TileContext(nc) as tc, ExitStack() as ctx:
        singles = ctx.enter_context(tc.tile_pool(name="singles", bufs=1))
        tiny = ctx.enter_context(tc.tile_pool(name="tiny", bufs=1))

        # ================= load static data =================
        w_sb = singles.tile([D, 12, D], F32)          # 12 layer weights
        nc.sync.dma_start(out=w_sb,
                          in_=wsT_d[:, :, :].rearrange("l a b -> a l b"))
        b_sb = singles.tile([D, 12], F32)             # bias columns
        nc.sync.dma_start(out=b_sb, in_=bsT_d[:, :].rearrange("l o -> o l"))
        wo_sb = singles.tile([D, 4], F32)
        nc.sync.dma_start(out=wo_sb, in_=woT_d[:, :].rearrange("h d -> d h"))
        bor_sb = singles.tile([1, 4 * D], F32)        # bo broadcast row
        nc.sync.dma_start(out=bor_sb, in_=bor_d[:, :])
        xT_sb = singles.tile([D, E], F32)
        nc.sync.dma_start(out=xT_sb, in_=xT_d[:, :])
        eband_sb = singles.tile([128, 127], BF16)
        nc.sync.dma_start(out=eband_sb, in_=eband_d[:, :])

        # ================= MLP =================
        with tc.tile_pool(name="mlp_ps", bufs=2, space="PSUM") as mlp_ps, \
             tc.tile_pool(name="mlp_sb", bufs=3) as mlp_sb, \
             tc.tile_pool(name="sc_ps", bufs=1, space="PSUM") as sc_ps:
            scores_ps = sc_ps.tile([1, 4 * D], F32)
            for h in range(4):
                hT = xT_sb
                for l in range(3):
                    li = h * 3 + l
                    p = mlp_ps.tile([D, D], F32, tag="mlp_p")
                    nc.tensor.matmul(p, w_sb[:, li, :], hT[:, :],
                                     start=True, stop=True)
                    yb = mlp_sb.tile([D, E], F32, tag="mlp_y")
                    nc.vector.tensor_scalar_add(yb, p, b_sb[:, li:li + 1])
                    hn = mlp_sb.tile([D, E], F32, tag="mlp_h")
                    # leaky_relu(y, 0.2) = max(0.2*y, y)
                    nc.vector.scalar_tensor_tensor(hn, yb, 0.2, yb,
                                                   op0=OP.mult, op1=OP.max)
                    if debug and h == 0 and l == 0:
                        nc.sync.dma_start(out=dbg["dbg_h0"][:, :], in_=hn)
                        nc.sync.dma_start(out=dbg["dbg_w0"][:, :],
                                          in_=w_sb[:, 0, :])
                    hT = hn
                # head final: lhsT = wo column [D,1], rhs = hT -> [1, E]
                nc.tensor.matmul(scores_ps[0:1, ts(h, D)],
                                 wo_sb[:, h:h + 1], hT[:, :],
                                 start=True, stop=True)

            # scores row (lane 0): s_all = scores + bo_bcast
            s_row = tiny.tile([1, 4 * D], F32)
            nc.vector.tensor_add(s_row, scores_ps, bor_sb)

        # ================= constants (all on lane 0) =================
        # sigma via the tanh table (4-ULP budget; the exp table is only
        # ~1e-5 relative, fatal for freq): sigma(x) = 0.5 + 0.5*tanh(x/2)
        th_row = tiny.tile([1, 4 * D], F32)
        nc.scalar.activation(th_row, s_row, A.Tanh, bias=0.0, scale=0.5)
        sig = tiny.tile([1, 4 * D], F32)
        nc.vector.tensor_scalar(sig, th_row, 0.5, 0.5,
                                op0=OP.mult, op1=OP.add)

        loc = sig[0:1, ts(0, D)]
        sv = sig[0:1, ts(1, D)]
        s2 = sig[0:1, ts(2, D)]
        amp = sig[0:1, ts(3, D)]

        cst = tiny.tile([1, 16 * D], F32)   # fp32 scratch rows, 128 each

        def R(i):
            return cst[0:1, ts(i, D)]

        var = R(0)
        nc.vector.tensor_scalar_add(var, sv, 1e-8)
        inv_var = R(1)
        nc.vector.reciprocal(inv_var, var)
        freq = R(2)
        nc.vector.tensor_mul(freq, s2, s2)
        p_row = R(3)                      # fl(freq*pi) — matches reference
        nc.vector.tensor_scalar_mul(p_row, freq, float(np.float32(np.pi)))

        a_c = R(4)
        nc.vector.tensor_scalar_mul(a_c, inv_var, INV_NM1)
        lv = R(5)                         # loc*inv_var
        nc.vector.tensor_mul(lv, loc, inv_var)
        b_c = R(6)
        nc.vector.tensor_scalar_mul(b_c, lv, -1.0)
        a2r = R(7)
        nc.vector.tensor_mul(a2r, a_c, a_c)
        abr = R(8)
        nc.vector.tensor_mul(abr, a_c, b_c)
        ab2r = R(9)
        nc.vector.tensor_scalar_mul(ab2r, abr, 2.0)
        b2r = R(10)
        nc.vector.tensor_mul(b2r, b_c, b_c)
        cw = R(11)                        # amp*inv_var/sqrt(2pi)
        nc.vector.tensor_mul(cw, amp, inv_var)
        nc.vector.tensor_scalar_mul(cw, cw, INV_SQRT_2PI)

        if debug:
            nc.sync.dma_start(out=dbg["dbg_scores"][:, :], in_=s_row)
            nc.sync.dma_start(out=dbg["dbg_sig"][:, :], in_=sig)

        tf1 = R(12)
        tf2 = R(13)
        tf3 = R(14)

        # ---- bf16 piece extraction into staging rows (all lane 0) ----
        # s pieces: s ~= fl(freq*pi) * (1/2pi) in extended precision
        sa_b = tiny.tile([1, 3 * D], BF16)   # s_a, s_b, s_c
        t1 = R(15)
        nc.vector.tensor_scalar_mul(t1, p_row, INV_2PI_HI)
        nc.vector.tensor_copy(sa_b[0:1, ts(0, D)], t1)            # s_a
        nc.vector.tensor_sub(tf1, t1, sa_b[0:1, ts(0, D)])        # r
        nc.vector.tensor_scalar_mul(tf2, p_row, INV_2PI_LO)
        nc.vector.tensor_add(tf2, tf2, tf1)                       # u2
        nc.vector.tensor_copy(sa_b[0:1, ts(1, D)], tf2)           # s_b
        nc.vector.tensor_sub(tf3, tf2, sa_b[0:1, ts(1, D)])
        nc.vector.tensor_copy(sa_b[0:1, ts(2, D)], tf3)           # s_c

        # stageA bf16 [1, 12*128]: 6 negative then 6 positive coef rows.
        stageA = tiny.tile([1, 12 * D], BF16)
        stageB = tiny.tile([1, 12 * D], BF16)

        m_coefs = [(256.0, 0), (256.0, 1), (256.0, 2),
                   (1.0, 0), (1.0, 1), (1.0, 2)]
        for r, (mul, piece) in enumerate(m_coefs):
            nc.vector.tensor_scalar_mul(stageA[0:1, ts(r, D)],
                                        sa_b[0:1, ts(piece, D)], -mul)
            nc.vector.tensor_scalar_mul(stageA[0:1, ts(6 + r, D)],
                                        sa_b[0:1, ts(piece, D)], mul)

        # z^2 coefficient pieces (2 bf16 pieces each for a2, ab2, b2)
        pz = tiny.tile([1, 6 * D], BF16)

        def split2_to(src, dst0, dst1):
            nc.vector.tensor_copy(dst0, src)
            nc.vector.tensor_sub(tf1, src, dst0)
            nc.vector.tensor_copy(dst1, tf1)

        split2_to(a2r, pz[0:1, ts(0, D)], pz[0:1, ts(1, D)])
        split2_to(ab2r, pz[0:1, ts(2, D)], pz[0:1, ts(3, D)])
        split2_to(b2r, pz[0:1, ts(4, D)], pz[0:1, ts(5, D)])

        bspec = [(1.0, 0), (1.0, 1), (1.0, 0), (1.0, 1), (1.0, 0), (1.0, 1),
                 (256.0, 2), (256.0, 3), (1.0, 2), (1.0, 3),
                 (1.0, 4), (1.0, 5)]
        for r, (mul, piece) in enumerate(bspec):
            dst = stageB[0:1, ts(r, D)]
            if mul == 1.0:
                nc.vector.tensor_copy(dst, pz[0:1, ts(piece, D)])
            else:
                nc.vector.tensor_scalar_mul(dst, pz[0:1, ts(piece, D)], mul)

        # ---- scatter staging rows to lhsT tiles via DRAM bounce ----
        lhsA1 = singles.tile([6, D], BF16)      # -m piece coefs
        lhsA2 = singles.tile([6, D], BF16)      # +m piece coefs
        lhsCn = singles.tile([1, D], BF16)      # -C row
        lhsCp = singles.tile([1, D], BF16)      # +C row
        ones_row = singles.tile([1, NT], BF16)  # rhs for the C matmuls
        lhsB = singles.tile([12, D], BF16)
        cw_col = singles.tile([D, 1], F32)

        nc.vector.memset(lhsCn, -C_MAGIC)
        nc.vector.memset(lhsCp, C_MAGIC)
        nc.vector.memset(ones_row, 1.0)

        nc.sync.dma_start(out=scrA_d[:, :], in_=stageA[:, :])
        nc.sync.dma_start(
            out=lhsA1,
            in_=scrA_d[0, 0:6 * D].rearrange("(r d) -> r d", d=D))
        nc.sync.dma_start(
            out=lhsA2,
            in_=scrA_d[0, 6 * D:12 * D].rearrange("(r d) -> r d", d=D))
        nc.sync.dma_start(out=scrB_d[:, :], in_=stageB[:, :])
        nc.sync.dma_start(out=lhsB,
                          in_=scrB_d[0, :].rearrange("(r d) -> r d", d=D))
        nc.sync.dma_start(out=scrC_d[:, :], in_=cw)
        nc.sync.dma_start(out=cw_col,
                          in_=scrC_d[0, :].rearrange("(d o) -> d o", o=1))

        if debug:
            nc.sync.dma_start(out=dbg["dbg_cst"][:, :], in_=cst)
            nc.sync.dma_start(out=dbg["dbg_lhsA1"][:, :], in_=lhsA1)
            nc.sync.dma_start(out=dbg["dbg_lhsA2"][:, :], in_=lhsA2)
            nc.sync.dma_start(out=dbg["dbg_lhsB"][:, :], in_=lhsB)
            nc.sync.dma_start(out=dbg["dbg_cwcol"][:, :], in_=cw_col)

        # ================= Phase A: sines =================
        sin_buf = singles.tile([E, N], BF16)       # 64KB/partition
        with tc.tile_pool(name="pa_ps", bufs=2, space="PSUM") as pa_ps, \
             tc.tile_pool(name="pa_sb", bufs=3) as pa_sb:
            for t in range(T):
                bm = pa_sb.tile([6, NT], BF16, tag="basisM")
                nc.sync.dma_start(out=bm, in_=basisM_d[:, ts(t, NT)])
                pm = pa_ps.tile([E, NT], F32, tag="pm")
                nc.tensor.matmul(pm, lhsA1, bm, start=True, stop=False,
                                 skip_group_check=True)
                nc.tensor.matmul(pm, lhsCn, ones_row, start=False,
                                 stop=False, skip_group_check=True)
                nc.tensor.matmul(pm, lhsCp, ones_row, start=False,
                                 stop=False, skip_group_check=True)
                nc.tensor.matmul(pm, lhsA2, bm, start=False, stop=True,
                                 skip_group_check=True)
                nc.scalar.activation(sin_buf[:, ts(t, NT)], pm, A.Sin,
                                     bias=0.0, scale=TWO_PI)
                if debug and t == 0:
                    pmc = pa_sb.tile([E, NT], F32, tag="dbg_pmc")
                    nc.vector.tensor_copy(pmc, pm)
                    nc.sync.dma_start(out=dbg["dbg_pm"][:, :], in_=pmc)
                    nc.sync.dma_start(out=dbg["dbg_sin"][:, :],
                                      in_=sin_buf[:, ts(0, NT)])

        # ================= Phase B: bumps, prod, event-sum =================
        with tc.tile_pool(name="pb_ps", bufs=2, space="PSUM") as pb_ps, \
             tc.tile_pool(name="po_ps", bufs=1, space="PSUM") as po_ps, \
             tc.tile_pool(name="pb_sb", bufs=3) as pb_sb:
            out_ps = po_ps.tile([T, NT], F32)
            for t in range(T):
                bb = pb_sb.tile([12, NT], BF16, tag="basisB")
                nc.sync.dma_start(out=bb, in_=basisB_d[:, ts(t, NT)])
                pz2 = pb_ps.tile([E, NT], F32, tag="pz")
                nc.tensor.matmul(pz2, lhsB, bb, start=True, stop=True)
                bump = pb_sb.tile([E, NT], BF16, tag="bump")
                nc.scalar.activation(bump, pz2, A.Exp, bias=0.0, scale=-0.5)
                prod = pb_sb.tile([E, NT], BF16, tag="prod")
                nc.vector.scalar_tensor_tensor(
                    prod, bump, cw_col[:, :], sin_buf[:, ts(t, NT)],
                    op0=OP.mult, op1=OP.mult)
                if debug and t == 0:
                    z2c = pb_sb.tile([E, NT], F32, tag="dbg_z2c")
                    nc.vector.tensor_copy(z2c, pz2)
                    nc.sync.dma_start(out=dbg["dbg_z2"][:, :], in_=z2c)
                    nc.sync.dma_start(out=dbg["dbg_bump"][:, :], in_=bump)
                    nc.sync.dma_start(out=dbg["dbg_prod"][:, :], in_=prod)
                nc.tensor.matmul(out_ps, eband_sb[:, 63 - t:127 - t], prod,
                                 start=(t == 0), stop=(t == T - 1),
                                 skip_group_check=True)

            # copy accumulated output out of PSUM before the pool closes
            out_sb = tiny.tile([T, NT], F32)
            nc.vector.tensor_copy(out_sb, out_ps)
            if debug:
                nc.sync.dma_start(out=dbg["dbg_outsb"][:, :], in_=out_sb)

        # ================= finalize: max-norm =================
        m1 = tiny.tile([T, 1], F32)
        nc.vector.tensor_reduce(m1, out_sb, axis=mybir.AxisListType.X,
                                op=OP.max, apply_absolute_value=True)
        nc.sync.dma_start(out=scrM_d[:, :], in_=m1)
        m1r = tiny.tile([1, T], F32)
        nc.sync.dma_start(out=m1r,
                          in_=scrM_d[:, 0].rearrange("(o t) -> o t", o=1))
        m2 = tiny.tile([1, 1], F32)
        nc.vector.tensor_reduce(m2, m1r, axis=mybir.AxisListType.X, op=OP.max)
        nc.vector.tensor_scalar_add(m2, m2, 1e-12)
        inv = tiny.tile([1, 1], F32)
        nc.vector.reciprocal(inv, m2)
        nc.sync.dma_start(out=scrI_d[:, :], in_=inv)
        invb = tiny.tile([T, 1], F32)
        invb_src = bass.AP(tensor=scrI_d, offset=0, ap=[[0, T], [1, 1]])
        nc.sync.dma_start(out=invb, in_=invb_src)
        out_n = tiny.tile([T, NT], F32)
        nc.vector.tensor_scalar_mul(out_n, out_sb, invb[:, :])
        nc.sync.dma_start(out=out_d[:, :], in_=out_n)

    return nc


def _legalize_sync(nc):
    """Split multi-wait instructions into single-wait NOP prefixes.

    This container's walrus encodes exactly one sem wait per instruction
    ("Too many sync wait commands" otherwise); Tile's kernel-tail drain
    aggregates one wait per live semaphore. Hoisting the extra waits onto
    same-engine NOPs immediately before the instruction is semantically
    identical (engines execute their instruction stream in order)."""
    from concourse import mybir

    n = 0
    for func in nc.m.functions:
        for block in func.blocks:
            out = []
            changed = False
            for inst in block.instructions:
                si = inst.sync_info
                if si is not None and len(si.on_wait) > 1:
                    waits = list(si.on_wait)
                    for w in waits[:-1]:
                        n += 1
                        nop = mybir.InstNoOp(name=f"lgl_wait_{n}")
                        nop.engine = inst.engine
                        nop.sync_info = mybir.SyncInfo(on_wait=[w],
                                                       on_update=[])
                        out.append(nop)
                    si.on_wait = [waits[-1]]
                    changed = True
                out.append(inst)
            if changed:
                block.instructions = out
    return n


def _get_nc():
    if "nc" not in _cached:
        nc = _build()
        _legalize_sync(nc)
        _cached["nc"] = nc
    return _cached["nc"]


def kernel(x, Ws, bs, Wo, bo):
    from concourse.bass_utils import run_bass_kernel_spmd

    x = np.asarray(x, np.float32).reshape(NB, E, D)
    Ws = np.asarray(Ws, np.float32)    # [4, 3, D, D]
    bs = np.asarray(bs, np.float32)    # [4, 3, D]
    Wo = np.asarray(Wo, np.float32)    # [4, 1, D]
    bo = np.asarray(bo, np.float32)    # [4, 1]

    # host-side input prep (layouts only)
    wsT = np.ascontiguousarray(Ws.reshape(12, D, D).transpose(0, 2, 1))
    bsT = np.ascontiguousarray(bs.reshape(12, D))
    woT = np.ascontiguousarray(Wo[:, 0, :])
    bor = np.ascontiguousarray(np.repeat(bo[:, 0], D)[None, :].astype(np.float32))

    nc = _get_nc()
    in_maps = []
    for b in range(NB):
        in_maps.append({
            "xT": np.ascontiguousarray(x[b].T),
            "wsT": wsT, "bsT": bsT, "woT": woT, "bor": bor,
        })
    res = run_bass_kernel_spmd(nc, in_maps, core_ids=list(range(NB)),
                               **_cached.get("run_kwargs", {}))
    kernel.last_results = res
    outs = [res.results[b]["out"].reshape(1, N) for b in range(NB)]
    return np.stack(outs).astype(np.float32)



# revision 6
# speedup vs baseline: 2.1032x; 2.1032x over previous
"""Trainium2 Bass kernel for nn_Decoder_67705864454693 (v2).

Module: 4-head LinearOutputStack MLP (loc/var/freq/amp per event) ->
sum_e amp*sin(freq*pi*n)*NormalPDF(loc,var)(rng[n]) over n=1..32768 -> max-norm.

Sharding: data-parallel over batch B=8, one batch per NeuronCore.

v2 redesign (v1 baseline: 266us, PE-bound at 189us of cold matmuls).
v2 removes all per-tile transcendental-argument matmuls and the per-tile
ACT sin via angle addition, collapsing to ONE fused hot loop and ONE ACT
table switch:

  phase(e, n) mod 1 = uM0(e, j) + u_c(e, t),  n = 512 t + j
    uM0  = frac(s*(j+1))   [128,512]  (one matmul + round-trick, once)
    u_c  = frac(s*512 t)   [128,64]   (PE magic-C trick, once)
  sin(2 pi phase) = sinM0*cos(2 pi u_c) + cosM0*sin(2 pi u_c)
  The [128,512] sin/cos tables are pre-multiplied by the chunk-periodic
  Gaussian factor G'(e, j mod 2048) = cw * exp(-(alpha*j)^2/2), giving
  [128,2048] fp16 tables; per tile the sine costs two 4x-mode
  tensor_scalars (per-event cos/sin columns) and one 2x tensor_tensor.

  bump: exp(-z^2/2) with z = alpha*n - beta factors per 2048-chunk as
  G' * E1, where E1 = Exp(scale_eT*j + bias_eT) is one ACT instruction
  per chunk (per-partition affine: scale = -alpha*z0, bias = -z0^2/2).

  event sum: ones-band matmul into PSUM rows (only hot-loop PE work).

Hot loop per 2048-chunk: 8 ts + 4 tt + 1 tt (DVE), 1 ACT exp, 4 matmuls.
"""
import numpy as np
import ml_dtypes

bfnp = ml_dtypes.bfloat16

N = 32768
E = 128
D = 128
NT = 512            # sample tile (matmul / table period)
T = N // NT         # 64
CH = 2048           # chunk (ACT / Gaussian factorization period)
TC = N // CH        # 16
QP = CH // NT       # 4 tiles per chunk
C_MAGIC = 12582912.0
NB = 8

_cached = {}


def _make_const():
    j = np.arange(NT, dtype=np.int64)
    jp = j + 1                                  # 1..512
    jh = (jp // 256).astype(np.float32)         # 0..2  bf16-exact
    jl = (jp % 256).astype(np.float32)          # 0..255 bf16-exact
    basisA = np.stack([jh, jh, jh, jl, jl, jl]).astype(bfnp)     # [6, 512]

    t2 = (2 * np.arange(T, dtype=np.int64)).astype(np.float32)   # 0..126
    tramp2 = np.stack([t2, t2, t2]).astype(bfnp)                 # [3, 64]

    Tramp = np.broadcast_to(np.arange(TC, dtype=np.float32),
                            (128, TC)).copy()                    # [128, 16]
    jc = np.arange(CH, dtype=np.float64)
    jramp = np.broadcast_to(jc, (128, CH)).astype(np.float16)    # [128, 2048]
    j2ramp = np.broadcast_to(jc * jc, (128, CH)).astype(bfnp)    # [128, 2048]

    eband = np.zeros((128, 127), np.float32)
    eband[:, 63] = 1.0

    eye = np.eye(128, dtype=np.float32)
    return basisA, tramp2, Tramp, jramp, j2ramp, eband.astype(bfnp), eye


def _build(debug=False):
    from contextlib import ExitStack
    import concourse.bass as bass
    import concourse.tile as tile
    from concourse import mybir
    from concourse.bass import ts

    F32 = mybir.dt.float32
    BF16 = mybir.dt.bfloat16
    FP16 = mybir.dt.float16
    A = mybir.ActivationFunctionType
    OP = mybir.AluOpType

    nc = bass.Bass()

    # ---- I/O ----
    xT_d = nc.dram_tensor("xT", [D, E], F32, kind="ExternalInput")
    ws_d = nc.dram_tensor("ws", [D, 12 * D], F32, kind="ExternalInput")
    bs_d = nc.dram_tensor("bs", [D, 12], F32, kind="ExternalInput")
    wo_d = nc.dram_tensor("wo", [D, 4], F32, kind="ExternalInput")
    bo_d = nc.dram_tensor("bo", [D, 4], F32, kind="ExternalInput")
    out_d = nc.dram_tensor("out", [T, NT], F32, kind="ExternalOutput")

    basisA_np, tramp2_np, Tramp_np, jramp_np, j2_np, eband_np, eye_np = \
        _make_const()
    basisA_d = nc.inline_tensor(basisA_np, name="basisA")
    tramp2_d = nc.inline_tensor(tramp2_np, name="tramp2")
    Tramp_d = nc.inline_tensor(Tramp_np, name="Tramp")
    jramp_d = nc.inline_tensor(jramp_np, name="jramp")
    j2_d = nc.inline_tensor(j2_np, name="j2ramp")
    eband_d = nc.inline_tensor(eband_np, name="eband")
    eye_d = nc.inline_tensor(eye_np, name="eye")

    PI_F32 = float(np.float32(np.pi))
    TWO_PI = float(2.0 * np.pi)
    INV_2PI_HI = float(np.float32(1.0 / (2.0 * np.pi)))
    INV_2PI_LO = float(np.float32(
        1.0 / (2.0 * np.pi) - np.float64(np.float32(1.0 / (2.0 * np.pi)))))
    INV_NM1 = float(np.float32(1.0 / (N - 1)))
    INV_SQRT_2PI = float(np.float32(1.0 / np.sqrt(2.0 * np.pi)))

    dbg = {}
    if debug:
        for nm, shape, dt in [
            ("dbg_sig", [128, 4], F32), ("dbg_cols", [128, 16], F32),
            ("dbg_lhs12", [12, 128], BF16), ("dbg_AB", [128, 2 * T], F32),
            ("dbg_um0", [128, NT], F32), ("dbg_sinm0", [128, NT], FP16),
            ("dbg_cosm0", [128, NT], FP16), ("dbg_gp", [128, CH], FP16),
            ("dbg_e1", [128, CH], FP16), ("dbg_sch", [128, CH], FP16),
            ("dbg_p2", [128, CH], BF16),
            ("dbg_scale", [128, TC], F32), ("dbg_bias", [128, TC], F32),
            ("dbg_outps", [T, NT], F32),
        ]:
            dbg[nm] = nc.dram_tensor(nm, shape, dt, kind="ExternalOutput")

    with tile.TileContext(nc) as tc, ExitStack() as ctx:
        singles = ctx.enter_context(tc.tile_pool(name="singles", bufs=1))

        # ---------- load static data ----------
        w_sb = singles.tile([D, 12, D], F32)
        nc.sync.dma_start(out=w_sb,
                          in_=ws_d[:, :].rearrange("a (l b) -> a l b", b=D))
        b_sb = singles.tile([D, 12], F32)
        nc.sync.dma_start(out=b_sb, in_=bs_d[:, :])
        wo_sb = singles.tile([D, 4], F32)
        nc.sync.dma_start(out=wo_sb, in_=wo_d[:, :])
        bo_sb = singles.tile([D, 4], F32)
        nc.sync.dma_start(out=bo_sb, in_=bo_d[:, :])
        xT_sb = singles.tile([D, E], F32)
        nc.sync.dma_start(out=xT_sb, in_=xT_d[:, :])
        basisA_sb = singles.tile([6, NT], BF16)
        nc.sync.dma_start(out=basisA_sb, in_=basisA_d[:, :])
        tramp2_sb = singles.tile([35, T], BF16)
        nc.sync.dma_start(out=tramp2_sb[0:3, :], in_=tramp2_d[:, :])
        nc.sync.dma_start(out=tramp2_sb[32:35, :], in_=tramp2_d[:, :])
        Tramp_sb = singles.tile([128, TC], F32)
        nc.sync.dma_start(out=Tramp_sb, in_=Tramp_d[:, :])
        jramp_sb = singles.tile([128, CH], FP16)
        nc.sync.dma_start(out=jramp_sb, in_=jramp_d[:, :])
        j2_sb = singles.tile([128, CH], BF16)
        nc.sync.dma_start(out=j2_sb, in_=j2_d[:, :])
        eband_sb = singles.tile([128, 127], BF16)
        nc.sync.dma_start(out=eband_sb, in_=eband_d[:, :])
        eye_sb = singles.tile([128, 128], F32)
        nc.sync.dma_start(out=eye_sb, in_=eye_d[:, :])

        ones64_sb = singles.tile([1, T], BF16)
        nc.vector.memset(ones64_sb, 1.0)
        lhsCn = singles.tile([1, D], BF16)
        nc.vector.memset(lhsCn, -C_MAGIC)
        lhsCp = singles.tile([1, D], BF16)
        nc.vector.memset(lhsCp, C_MAGIC)
        ones1_sb = singles.tile([1, T], F32)
        nc.vector.memset(ones1_sb, 1.0)

        M0_sb = singles.tile([128, NT], F32)
        lhs36_sb = singles.tile([36, 128], BF16)
        A64_sb = singles.tile([128, T], F32)      # sin(2pi u_c)
        B64_sb = singles.tile([128, T], F32)      # cos(2pi u_c)
        sinG_sb = singles.tile([128, CH], FP16)   # sin(2pi uM0)*G' (4x tiled)
        cosG_sb = singles.tile([128, CH], FP16)
        scaleT_sb = singles.tile([128, TC], F32)
        biasT_sb = singles.tile([128, TC], F32)

        # ---------- MLP (fp32; scores come out as COLUMNS) ----------
        sig4 = singles.tile([128, 4], F32)
        with tc.tile_pool(name="mlp_ps", bufs=2, space="PSUM") as mlp_ps, \
             tc.tile_pool(name="sc_ps", bufs=1, space="PSUM") as sc_ps, \
             tc.tile_pool(name="mlp_sb", bufs=3) as mlp_sb:
            scores_ps = sc_ps.tile([128, 4], F32)
            for h in range(4):
                hT = xT_sb
                for l in range(3):
                    li = h * 3 + l
                    p = mlp_ps.tile([D, E], F32, tag="mlp_p")
                    nc.tensor.matmul(p, w_sb[:, li, :], hT[:, :],
                                     start=True, stop=True)
                    yb = mlp_sb.tile([D, E], F32, tag="mlp_y")
                    nc.vector.tensor_scalar_add(yb, p, b_sb[:, li:li + 1])
                    hn = mlp_sb.tile([D, E], F32, tag="mlp_h")
                    nc.vector.scalar_tensor_tensor(hn, yb, 0.2, yb,
                                                   op0=OP.mult, op1=OP.max)
                    hT = hn
                # scores column for head h: lhsT=hT [D,E], rhs=wo col [D,1]
                nc.tensor.matmul(scores_ps[:, h:h + 1], hT[:, :],
                                 wo_sb[:, h:h + 1], start=True, stop=True)

            s4 = mlp_sb.tile([128, 4], F32, tag="s4")
            nc.vector.tensor_add(s4, scores_ps, bo_sb[:, 0:4])
            # sigmoid via tanh table: sig = 0.5 + 0.5*tanh(x/2)
            th4 = mlp_sb.tile([128, 4], F32, tag="th4")
            nc.scalar.activation(th4, s4, A.Tanh, bias=0.0, scale=0.5)
            nc.vector.tensor_scalar(sig4, th4, 0.5, 0.5,
                                    op0=OP.mult, op1=OP.add)

        loc = sig4[:, 0:1]
        sv = sig4[:, 1:2]
        s2 = sig4[:, 2:3]
        amp = sig4[:, 3:4]

        # ---------- per-event scalar columns ----------
        cst = singles.tile([128, 16], F32)

        def R(i):
            return cst[:, i:i + 1]

        var = R(0)
        nc.vector.tensor_scalar_add(var, sv, 1e-8)
        ivar = R(1)
        nc.vector.reciprocal(ivar, var)
        freq = R(2)
        nc.vector.tensor_mul(freq, s2, s2)
        p_c = R(3)                       # fl(freq*pi) - must match reference
        nc.vector.tensor_scalar_mul(p_c, freq, PI_F32)

        # s = p/(2pi) in 3 bf16 pieces (extended precision)
        sab = singles.tile([128, 3], BF16)
        t1 = R(4)
        nc.vector.tensor_scalar_mul(t1, p_c, INV_2PI_HI)
        nc.vector.tensor_copy(sab[:, 0:1], t1)
        r1 = R(5)
        nc.vector.tensor_sub(r1, t1, sab[:, 0:1])
        t2 = R(6)
        nc.vector.tensor_scalar_mul(t2, p_c, INV_2PI_LO)
        nc.vector.tensor_add(t2, t2, r1)
        nc.vector.tensor_copy(sab[:, 1:2], t2)
        r2 = R(7)
        nc.vector.tensor_sub(r2, t2, sab[:, 1:2])
        nc.vector.tensor_copy(sab[:, 2:3], r2)

        # scol36 [128,36]: cols 0-5 = [256sa,256sb,256sc, sa,sb,sc];
        # cols 32-34 = [-256sa,-256sb,-256sc] (32-aligned so the transposed
        # rows slice at base partition 32 for the u_c tree matmul).
        scol36 = singles.tile([128, 36], BF16)
        nc.vector.memset(scol36, 0.0)
        for i in range(3):
            nc.vector.tensor_scalar_mul(scol36[:, i:i + 1], sab[:, i:i + 1],
                                        256.0)
            nc.vector.tensor_copy(scol36[:, 3 + i:4 + i], sab[:, i:i + 1])
            nc.vector.tensor_scalar_mul(scol36[:, 32 + i:33 + i],
                                        sab[:, i:i + 1], -256.0)

        # Gaussian columns
        alpha = R(8)
        nc.vector.tensor_scalar_mul(alpha, ivar, INV_NM1)
        nalpha = R(9)
        nc.vector.tensor_scalar_mul(nalpha, alpha, -1.0)
        beta = R(10)
        nc.vector.tensor_mul(beta, loc, ivar)
        cw = R(11)                       # amp*ivar/sqrt(2pi)
        nc.vector.tensor_mul(cw, amp, ivar)
        nc.vector.tensor_scalar_mul(cw, cw, INV_SQRT_2PI)
        a2048 = R(12)
        nc.vector.tensor_scalar_mul(a2048, alpha, float(CH))
        na2h = R(13)                     # -alpha^2/2
        nc.vector.tensor_mul(na2h, alpha, alpha)
        nc.vector.tensor_scalar_mul(na2h, na2h, -0.5)

        # z0[e,T] = alpha*2048*T - beta ; scale = -alpha*z0 ; bias = -z0^2/2
        z0 = singles.tile([128, TC], F32)
        nc.vector.tensor_scalar(z0, Tramp_sb, a2048, beta,
                                op0=OP.mult, op1=OP.subtract)
        nc.vector.tensor_scalar(scaleT_sb, z0, nalpha, None, op0=OP.mult)
        z0sq = singles.tile([128, TC], F32)
        nc.vector.tensor_mul(z0sq, z0, z0)
        nc.vector.tensor_scalar_mul(biasT_sb, z0sq, -0.5)

        # ---------- transpose s-columns -> lhsT rows ----------
        with tc.tile_pool(name="tr_ps", bufs=1, space="PSUM") as tr_ps:
            tr36 = tr_ps.tile([36, 128], BF16)
            eye_bf = singles.tile([128, 128], BF16)
            nc.vector.tensor_copy(eye_bf, eye_sb)
            nc.tensor.transpose(tr36, scol36, eye_bf)
            nc.vector.tensor_copy(lhs36_sb, tr36)

        if debug:
            nc.sync.dma_start(out=dbg["dbg_sig"][:, :], in_=sig4)
            nc.sync.dma_start(out=dbg["dbg_cols"][:, :], in_=cst)
            nc.sync.dma_start(out=dbg["dbg_lhs12"][:, :],
                              in_=lhs36_sb[0:12, :])
            nc.sync.dma_start(out=dbg["dbg_scale"][:, :], in_=scaleT_sb)
            nc.sync.dma_start(out=dbg["dbg_bias"][:, :], in_=biasT_sb)

        # helper: centered frac via magic-C round trick (x -> x - round(x))
        def frac_center(out_ap, in_ap, rtile, op0_scalar=None):
            """rtile: scratch F32 tile same shape; out = in - round(in)."""
            if op0_scalar is None:
                nc.vector.tensor_scalar(rtile, in_ap, C_MAGIC, C_MAGIC,
                                        op0=OP.add, op1=OP.subtract)
            else:
                # out = (in + s) - round(in + s): first materialize in+s
                raise NotImplementedError
            nc.vector.tensor_sub(out_ap, in_ap, rtile)

        # ---------- u_c tables: A64 = sin(2pi u_c), B64 = cos(2pi u_c) ----
        with tc.tile_pool(name="uc_ps", bufs=1, space="PSUM") as uc_ps, \
             tc.tile_pool(name="uc_sb", bufs=1) as uc_sb:
            ucp = uc_ps.tile([128, T], F32)
            nc.tensor.matmul(ucp, lhs36_sb[32:35, :], tramp2_sb[32:35, :],
                             start=True, stop=False, skip_group_check=True)
            nc.tensor.matmul(ucp, lhsCn, ones64_sb, start=False, stop=False,
                             skip_group_check=True)
            nc.tensor.matmul(ucp, lhsCp, ones64_sb, start=False, stop=False,
                             skip_group_check=True)
            nc.tensor.matmul(ucp, lhs36_sb[0:3, :], tramp2_sb[0:3, :],
                             start=False, stop=True, skip_group_check=True)
            # ucp = u_c in [-0.5, 0.5]
            nc.scalar.activation(A64_sb, ucp, A.Sin, bias=0.0, scale=TWO_PI)
            # cos via sin(2pi*(frac_center(u_c + 0.25)))
            uq = uc_sb.tile([128, T], F32, tag="uq")
            nc.vector.tensor_scalar_add(uq, ucp, 0.25)
            rq = uc_sb.tile([128, T], F32, tag="rq")
            uqf = uc_sb.tile([128, T], F32, tag="uqf")
            frac_center(uqf, uq, rq)
            nc.scalar.activation(B64_sb, uqf, A.Sin, bias=0.0, scale=TWO_PI)

        # ---------- uM0 tables: sinM0 / cosM0 [128,512] fp16 ----------
        sinM0 = singles.tile([128, NT], FP16)
        cosM0 = singles.tile([128, NT], FP16)
        with tc.tile_pool(name="m0_ps", bufs=1, space="PSUM") as m0_ps, \
             tc.tile_pool(name="m0_sb", bufs=1) as m0_sb:
            m0p = m0_ps.tile([128, NT], F32)
            nc.tensor.matmul(m0p, lhs36_sb[0:6, :], basisA_sb,
                             start=True, stop=True)
            nc.vector.tensor_copy(M0_sb, m0p)
            um0 = m0_sb.tile([128, NT], F32, tag="um0")
            rm0 = m0_sb.tile([128, NT], F32, tag="rm0")
            frac_center(um0, M0_sb, rm0)
            nc.scalar.activation(sinM0, um0, A.Sin, bias=0.0, scale=TWO_PI)
            uq0 = m0_sb.tile([128, NT], F32, tag="uq0")
            nc.vector.tensor_scalar_add(uq0, um0, 0.25)
            rq0 = m0_sb.tile([128, NT], F32, tag="rq0")
            uqf0 = m0_sb.tile([128, NT], F32, tag="uqf0")
            frac_center(uqf0, uq0, rq0)
            nc.scalar.activation(cosM0, uqf0, A.Sin, bias=0.0, scale=TWO_PI)
            if debug:
                nc.sync.dma_start(out=dbg["dbg_um0"][:, :], in_=um0)
                nc.sync.dma_start(out=dbg["dbg_sinm0"][:, :], in_=sinM0)
                nc.sync.dma_start(out=dbg["dbg_cosm0"][:, :], in_=cosM0)
                nc.sync.dma_start(
                    out=dbg["dbg_AB"][:, 0:T], in_=A64_sb)
                nc.sync.dma_start(
                    out=dbg["dbg_AB"][:, T:2 * T], in_=B64_sb)

        # ---------- G' table and G'-premultiplied sin/cos tables ----------
        # (first Exp switches the ACT table set; no sins after this point)
        gp = singles.tile([128, CH], FP16)
        graw = singles.tile([128, CH], BF16)
        nc.scalar.activation(graw, j2_sb, A.Exp, bias=0.0, scale=na2h)
        nc.vector.tensor_scalar(gp, graw, cw, None, op0=OP.mult)
        for q in range(QP):
            nc.vector.tensor_mul(sinG_sb[:, ts(q, NT)], sinM0,
                                 gp[:, ts(q, NT)])
            nc.vector.tensor_mul(cosG_sb[:, ts(q, NT)], cosM0,
                                 gp[:, ts(q, NT)])
        if debug:
            nc.sync.dma_start(out=dbg["dbg_gp"][:, :], in_=gp)

        # ---------- fused hot loop ----------
        with tc.tile_pool(name="hl_sb", bufs=2) as hl, \
             tc.tile_pool(name="po_ps", bufs=1, space="PSUM") as po_ps:
            out_ps = po_ps.tile([T, NT], F32)
            for c in range(TC):
                e1 = hl.tile([128, CH], FP16, tag="e1")
                nc.scalar.activation(e1, jramp_sb, A.Exp,
                                     bias=biasT_sb[:, c:c + 1],
                                     scale=scaleT_sb[:, c:c + 1])
                sch = hl.tile([128, CH], FP16, tag="sch")
                for q in range(QP):
                    t = QP * c + q
                    q1 = hl.tile([128, NT], FP16, tag="q1")
                    nc.vector.tensor_scalar(q1, sinG_sb[:, ts(q, NT)],
                                            B64_sb[:, t:t + 1], None,
                                            op0=OP.mult)
                    q2 = hl.tile([128, NT], FP16, tag="q2")
                    nc.vector.tensor_scalar(q2, cosG_sb[:, ts(q, NT)],
                                            A64_sb[:, t:t + 1], None,
                                            op0=OP.mult)
                    nc.vector.tensor_add(sch[:, ts(q, NT)], q1, q2)
                p2 = hl.tile([128, CH], BF16, tag="p2")
                nc.vector.tensor_mul(p2, e1, sch)
                if debug and c == 0:
                    nc.sync.dma_start(out=dbg["dbg_e1"][:, :], in_=e1)
                    nc.sync.dma_start(out=dbg["dbg_sch"][:, :], in_=sch)
                    nc.sync.dma_start(out=dbg["dbg_p2"][:, :], in_=p2)
                for q in range(QP):
                    t = QP * c + q
                    nc.tensor.matmul(out_ps, eband_sb[:, 63 - t:127 - t],
                                     p2[:, ts(q, NT)],
                                     start=(t == 0), stop=(t == T - 1),
                                     skip_group_check=True)

            # ---------- finalize: max-norm ----------
            if debug:
                outc = singles.tile([T, NT], F32)
                nc.vector.tensor_copy(outc, out_ps)
                nc.sync.dma_start(out=dbg["dbg_outps"][:, :], in_=outc)
            m1 = singles.tile([T, 1], F32)
            nc.vector.tensor_reduce(m1, out_ps, axis=mybir.AxisListType.X,
                                    op=OP.max, apply_absolute_value=True)
            with tc.tile_pool(name="fin_ps", bufs=1, space="PSUM") as fin_ps:
                m1t = fin_ps.tile([1, T], F32)
                nc.tensor.transpose(m1t, m1, eye_sb[0:T, 0:T])
                m1r = singles.tile([1, T], F32)
                nc.vector.tensor_copy(m1r, m1t)
                m2 = singles.tile([1, 1], F32)
                nc.vector.tensor_reduce(m2, m1r, axis=mybir.AxisListType.X,
                                        op=OP.max)
                nc.vector.tensor_scalar_add(m2, m2, 1e-12)
                inv = singles.tile([1, 1], F32)
                nc.vector.reciprocal(inv, m2)
                invb = fin_ps.tile([T, 1], F32)
                nc.tensor.matmul(invb, ones1_sb, inv, start=True, stop=True)
                inv64 = singles.tile([T, 1], F32)
                nc.vector.tensor_copy(inv64, invb)
                out_n = singles.tile([T, NT], F32)
                nc.vector.tensor_scalar(out_n, out_ps, inv64[:, :], None,
                                        op0=OP.mult)
                nc.sync.dma_start(out=out_d[:, :], in_=out_n)

    return nc


def _legalize_sync(nc):
    """Split multi-wait instructions into single-wait NOP prefixes."""
    from concourse import mybir

    n = 0
    for func in nc.m.functions:
        for block in func.blocks:
            out = []
            changed = False
            for inst in block.instructions:
                si = inst.sync_info
                if si is not None and len(si.on_wait) > 1:
                    waits = list(si.on_wait)
                    for w in waits[:-1]:
                        n += 1
                        nop = mybir.InstNoOp(name=f"lgl_wait_{n}")
                        nop.engine = inst.engine
                        nop.sync_info = mybir.SyncInfo(on_wait=[w],
                                                       on_update=[])
                        out.append(nop)
                    si.on_wait = [waits[-1]]
                    changed = True
                out.append(inst)
            if changed:
                block.instructions = out
    return n


def _get_nc():
    if "nc" not in _cached:
        nc = _build(debug=_cached.get("debug", False))
        _legalize_sync(nc)
        _cached["nc"] = nc
    return _cached["nc"]


def kernel(x, Ws, bs, Wo, bo):
    from concourse.bass_utils import run_bass_kernel_spmd

    x = np.asarray(x, np.float32).reshape(NB, E, D)
    Ws = np.asarray(Ws, np.float32)    # [4, 3, D, D]
    bs = np.asarray(bs, np.float32)    # [4, 3, D]
    Wo = np.asarray(Wo, np.float32)    # [4, 1, D]
    bo = np.asarray(bo, np.float32)    # [4, 1]

    # host-side layout prep (contiguous DMAs)
    ws_h = np.ascontiguousarray(
        Ws.reshape(12, D, D).transpose(2, 0, 1)).reshape(D, 12 * D)
    bs_h = np.ascontiguousarray(bs.reshape(12, D).T)        # [D, 12]
    wo_h = np.ascontiguousarray(Wo[:, 0, :].T)              # [D, 4]
    bo_h = np.ascontiguousarray(
        np.broadcast_to(bo[:, 0], (D, 4))).astype(np.float32)

    nc = _get_nc()
    in_maps = []
    for b in range(NB):
        in_maps.append({
            "xT": np.ascontiguousarray(x[b].T),
            "ws": ws_h, "bs": bs_h, "wo": wo_h, "bo": bo_h,
        })
    res = run_bass_kernel_spmd(nc, in_maps, core_ids=list(range(NB)),
                               **_cached.get("run_kwargs", {}))
    kernel.last_results = res
    outs = [res.results[b]["out"].reshape(1, N) for b in range(NB)]
    return np.stack(outs).astype(np.float32)


# revision 7
# speedup vs baseline: 3.1140x; 1.4806x over previous
"""Trainium2 Bass kernel for nn_Decoder_67705864454693 (v2).

Module: 4-head LinearOutputStack MLP (loc/var/freq/amp per event) ->
sum_e amp*sin(freq*pi*n)*NormalPDF(loc,var)(rng[n]) over n=1..32768 -> max-norm.

Sharding: data-parallel over batch B=8, one batch per NeuronCore.

v2 redesign (v1 baseline: 266us, PE-bound at 189us of cold matmuls).
v2 removes all per-tile transcendental-argument matmuls and the per-tile
ACT sin via angle addition, collapsing to ONE fused hot loop and ONE ACT
table switch:

  phase(e, n) mod 1 = uM0(e, j) + u_c(e, t),  n = 512 t + j
    uM0  = frac(s*(j+1))   [128,512]  (one matmul + round-trick, once)
    u_c  = frac(s*512 t)   [128,64]   (PE magic-C trick, once)
  sin(2 pi phase) = sinM0*cos(2 pi u_c) + cosM0*sin(2 pi u_c)
  The [128,512] sin/cos tables are pre-multiplied by the chunk-periodic
  Gaussian factor G'(e, j mod 2048) = cw * exp(-(alpha*j)^2/2), giving
  [128,2048] fp16 tables; per tile the sine costs two 4x-mode
  tensor_scalars (per-event cos/sin columns) and one 2x tensor_tensor.

  bump: exp(-z^2/2) with z = alpha*n - beta factors per 2048-chunk as
  G' * E1, where E1 = Exp(scale_eT*j + bias_eT) is one ACT instruction
  per chunk (per-partition affine: scale = -alpha*z0, bias = -z0^2/2).

  event sum: ones-band matmul into PSUM rows (only hot-loop PE work).

Hot loop per 2048-chunk: 8 ts + 4 tt + 1 tt (DVE), 1 ACT exp, 4 matmuls.
"""
import numpy as np
import ml_dtypes

bfnp = ml_dtypes.bfloat16

N = 32768
E = 128
D = 128
NT = 512            # sample tile (matmul / table period)
T = N // NT         # 64
CH = 2048           # chunk (ACT / Gaussian factorization period)
TC = N // CH        # 16
QP = CH // NT       # 4 tiles per chunk
C_MAGIC = 12582912.0
NB = 8

_cached = {}


def _make_const():
    j = np.arange(NT, dtype=np.int64)
    jp = j + 1                                  # 1..512
    jh = (jp // 256).astype(np.float32)         # 0..2  bf16-exact
    jl = (jp % 256).astype(np.float32)          # 0..255 bf16-exact
    basisA = np.stack([jh, jh, jh, jl, jl, jl]).astype(bfnp)     # [6, 512]

    t2 = (2 * np.arange(T, dtype=np.int64)).astype(np.float32)   # 0..126
    tramp2 = np.stack([t2, t2, t2]).astype(bfnp)                 # [3, 64]

    Tramp = np.broadcast_to(np.arange(TC, dtype=np.float32),
                            (128, TC)).copy()                    # [128, 16]
    jc = np.arange(CH, dtype=np.float64)
    jramp = np.broadcast_to(jc, (128, CH)).astype(np.float16)    # [128, 2048]
    j2ramp = np.broadcast_to(jc * jc, (128, CH)).astype(bfnp)    # [128, 2048]

    eye = np.eye(128, dtype=np.float32)
    return basisA, tramp2, Tramp, jramp, j2ramp, eye


def _build(debug=False):
    from contextlib import ExitStack
    import concourse.bass as bass
    import concourse.tile as tile
    from concourse import mybir
    from concourse.bass import ts

    F32 = mybir.dt.float32
    BF16 = mybir.dt.bfloat16
    FP16 = mybir.dt.float16
    A = mybir.ActivationFunctionType
    OP = mybir.AluOpType

    nc = bass.Bass()

    # ---- I/O ----
    xT_d = nc.dram_tensor("xT", [D, E], F32, kind="ExternalInput")
    ws_d = nc.dram_tensor("ws", [D, 12 * D], F32, kind="ExternalInput")
    bs_d = nc.dram_tensor("bs", [D, 12], F32, kind="ExternalInput")
    wo_d = nc.dram_tensor("wo", [D, 4], F32, kind="ExternalInput")
    bo_d = nc.dram_tensor("bo", [D, 4], F32, kind="ExternalInput")
    out_d = nc.dram_tensor("out", [T, NT], F32, kind="ExternalOutput")

    basisA_np, tramp2_np, Tramp_np, jramp_np, j2_np, eye_np = _make_const()
    basisA_d = nc.inline_tensor(basisA_np, name="basisA")
    tramp2_d = nc.inline_tensor(tramp2_np, name="tramp2")
    Tramp_d = nc.inline_tensor(Tramp_np, name="Tramp")
    jramp_d = nc.inline_tensor(jramp_np, name="jramp")
    j2_d = nc.inline_tensor(j2_np, name="j2ramp")
    eye_d = nc.inline_tensor(eye_np, name="eye")

    PI_F32 = float(np.float32(np.pi))
    TWO_PI = float(2.0 * np.pi)
    INV_2PI_HI = float(np.float32(1.0 / (2.0 * np.pi)))
    INV_2PI_LO = float(np.float32(
        1.0 / (2.0 * np.pi) - np.float64(np.float32(1.0 / (2.0 * np.pi)))))
    INV_NM1 = float(np.float32(1.0 / (N - 1)))
    INV_SQRT_2PI = float(np.float32(1.0 / np.sqrt(2.0 * np.pi)))

    dbg = {}
    if debug:
        for nm, shape, dt in [
            ("dbg_sig", [128, 4], F32), ("dbg_cols", [128, 16], F32),
            ("dbg_lhs12", [12, 128], BF16), ("dbg_AB", [128, 2 * T], F32),
            ("dbg_um0", [128, NT], F32), ("dbg_sinm0", [128, NT], FP16),
            ("dbg_cosm0", [128, NT], FP16), ("dbg_gp", [128, CH], FP16),
            ("dbg_e1", [128, CH], FP16), ("dbg_sch", [128, CH], FP16),
            ("dbg_p2", [128, CH], BF16),
            ("dbg_scale", [128, TC], F32), ("dbg_bias", [128, TC], F32),
            ("dbg_outps", [T, NT], F32),
        ]:
            dbg[nm] = nc.dram_tensor(nm, shape, dt, kind="ExternalOutput")

    with tile.TileContext(nc) as tc, ExitStack() as ctx:
        singles = ctx.enter_context(tc.tile_pool(name="singles", bufs=1))

        # ---------- load static data (MLP inputs first) ----------
        xT_sb = singles.tile([D, E], F32)
        nc.sync.dma_start(out=xT_sb, in_=xT_d[:, :])
        w_sb = singles.tile([D, 12, D], F32)
        nc.sync.dma_start(out=w_sb,
                          in_=ws_d[:, :].rearrange("a (l b) -> a l b", b=D))
        b_sb = singles.tile([D, 12], F32)
        nc.sync.dma_start(out=b_sb, in_=bs_d[:, :])
        wo_sb = singles.tile([D, 4], F32)
        nc.sync.dma_start(out=wo_sb, in_=wo_d[:, :])
        bo_sb = singles.tile([D, 4], F32)
        nc.sync.dma_start(out=bo_sb, in_=bo_d[:, :])
        basisA_sb = singles.tile([6, NT], BF16)
        nc.sync.dma_start(out=basisA_sb, in_=basisA_d[:, :])
        tramp2_sb = singles.tile([35, T], BF16)
        nc.sync.dma_start(out=tramp2_sb[0:3, :], in_=tramp2_d[:, :])
        nc.sync.dma_start(out=tramp2_sb[32:35, :], in_=tramp2_d[:, :])
        Tramp_sb = singles.tile([128, TC], F32)
        nc.sync.dma_start(out=Tramp_sb, in_=Tramp_d[:, :])
        jramp_sb = singles.tile([128, CH], FP16)
        nc.sync.dma_start(out=jramp_sb, in_=jramp_d[:, :])
        j2_sb = singles.tile([128, CH], BF16)
        nc.sync.dma_start(out=j2_sb, in_=j2_d[:, :])
        eye_sb = singles.tile([128, 128], F32)
        nc.sync.dma_start(out=eye_sb, in_=eye_d[:, :])

        ones64_sb = singles.tile([1, T], BF16)
        nc.vector.memset(ones64_sb, 1.0)
        lhsCn = singles.tile([1, D], BF16)
        nc.vector.memset(lhsCn, -C_MAGIC)
        lhsCp = singles.tile([1, D], BF16)
        nc.vector.memset(lhsCp, C_MAGIC)
        ones1_sb = singles.tile([1, T], F32)
        nc.vector.memset(ones1_sb, 1.0)

        M0_sb = singles.tile([128, NT], F32)
        lhs36_sb = singles.tile([36, 128], BF16)
        A64_sb = singles.tile([128, T], F32)      # sin(2pi u_c)
        B64_sb = singles.tile([128, T], F32)      # cos(2pi u_c)
        sinG_sb = singles.tile([128, CH], FP16)   # sin(2pi uM0)*G' (4x tiled)
        cosG_sb = singles.tile([128, CH], FP16)
        scaleT_sb = singles.tile([128, TC], F32)
        biasT_sb = singles.tile([128, TC], F32)

        # ---------- MLP (fp32; scores come out as COLUMNS) ----------
        sig4 = singles.tile([128, 4], F32)
        with tc.tile_pool(name="mlp_ps", bufs=2, space="PSUM") as mlp_ps, \
             tc.tile_pool(name="sc_ps", bufs=1, space="PSUM") as sc_ps, \
             tc.tile_pool(name="mlp_sb", bufs=3) as mlp_sb:
            scores_ps = sc_ps.tile([128, 4], F32)
            # layer-major interleave: the 4 heads' matmuls pipeline on the
            # PE while the DVE applies bias+lrelu of the previous head.
            hcur = [xT_sb] * 4
            for l in range(3):
                for h in range(4):
                    li = h * 3 + l
                    p = mlp_ps.tile([D, E], F32, tag=f"mlp_p{h}", bufs=1)
                    nc.tensor.matmul(p, w_sb[:, li, :], hcur[h][:, :],
                                     start=True, stop=True)
                    yb = mlp_sb.tile([D, E], F32, tag=f"mlp_y{h}", bufs=1)
                    nc.vector.tensor_scalar_add(yb, p, b_sb[:, li:li + 1])
                    hn = mlp_sb.tile([D, E], F32, tag=f"mlp_h{h}", bufs=2)
                    nc.vector.scalar_tensor_tensor(hn, yb, 0.2, yb,
                                                   op0=OP.mult, op1=OP.max)
                    hcur[h] = hn
            for h in range(4):
                nc.tensor.matmul(scores_ps[:, h:h + 1], hcur[h][:, :],
                                 wo_sb[:, h:h + 1], start=True, stop=True)

            s4 = mlp_sb.tile([128, 4], F32, tag="s4")
            nc.vector.tensor_add(s4, scores_ps, bo_sb[:, 0:4])
            # sigmoid via tanh table: sig = 0.5 + 0.5*tanh(x/2)
            th4 = mlp_sb.tile([128, 4], F32, tag="th4")
            nc.scalar.activation(th4, s4, A.Tanh, bias=0.0, scale=0.5)
            nc.vector.tensor_scalar(sig4, th4, 0.5, 0.5,
                                    op0=OP.mult, op1=OP.add)

        loc = sig4[:, 0:1]
        sv = sig4[:, 1:2]
        s2 = sig4[:, 2:3]
        amp = sig4[:, 3:4]

        # ---------- per-event scalar columns ----------
        cst = singles.tile([128, 16], F32)

        def R(i):
            return cst[:, i:i + 1]

        var = R(0)
        nc.vector.tensor_scalar_add(var, sv, 1e-8)
        ivar = R(1)
        nc.vector.reciprocal(ivar, var)
        freq = R(2)
        nc.vector.tensor_mul(freq, s2, s2)
        p_c = R(3)                       # fl(freq*pi) - must match reference
        nc.vector.tensor_scalar_mul(p_c, freq, PI_F32)

        # s = p/(2pi) in 3 bf16 pieces (extended precision)
        sab = singles.tile([128, 3], BF16)
        t1 = R(4)
        nc.vector.tensor_scalar_mul(t1, p_c, INV_2PI_HI)
        nc.vector.tensor_copy(sab[:, 0:1], t1)
        r1 = R(5)
        nc.vector.tensor_sub(r1, t1, sab[:, 0:1])
        t2 = R(6)
        nc.vector.tensor_scalar_mul(t2, p_c, INV_2PI_LO)
        nc.vector.tensor_add(t2, t2, r1)
        nc.vector.tensor_copy(sab[:, 1:2], t2)
        r2 = R(7)
        nc.vector.tensor_sub(r2, t2, sab[:, 1:2])
        nc.vector.tensor_copy(sab[:, 2:3], r2)

        # scol36 [128,36]: cols 0-5 = [256sa,256sb,256sc, sa,sb,sc];
        # cols 32-34 = [-256sa,-256sb,-256sc] (32-aligned so the transposed
        # rows slice at base partition 32 for the u_c tree matmul).
        scol36 = singles.tile([128, 36], BF16)
        nc.vector.memset(scol36, 0.0)
        for i in range(3):
            nc.vector.tensor_scalar_mul(scol36[:, i:i + 1], sab[:, i:i + 1],
                                        256.0)
            nc.vector.tensor_copy(scol36[:, 3 + i:4 + i], sab[:, i:i + 1])
            nc.vector.tensor_scalar_mul(scol36[:, 32 + i:33 + i],
                                        sab[:, i:i + 1], -256.0)

        # Gaussian columns
        alpha = R(8)
        nc.vector.tensor_scalar_mul(alpha, ivar, INV_NM1)
        nalpha = R(9)
        nc.vector.tensor_scalar_mul(nalpha, alpha, -1.0)
        beta = R(10)
        nc.vector.tensor_mul(beta, loc, ivar)
        cw = R(11)                       # amp*ivar/sqrt(2pi)
        nc.vector.tensor_mul(cw, amp, ivar)
        nc.vector.tensor_scalar_mul(cw, cw, INV_SQRT_2PI)
        a2048 = R(12)
        nc.vector.tensor_scalar_mul(a2048, alpha, float(CH))
        na2h = R(13)                     # -alpha^2/2
        nc.vector.tensor_mul(na2h, alpha, alpha)
        nc.vector.tensor_scalar_mul(na2h, na2h, -0.5)

        # z0[e,T] = alpha*2048*T - beta ; scale = -alpha*z0 ; bias = -z0^2/2
        z0 = singles.tile([128, TC], F32)
        nc.vector.tensor_scalar(z0, Tramp_sb, a2048, beta,
                                op0=OP.mult, op1=OP.subtract)
        nc.vector.tensor_scalar(scaleT_sb, z0, nalpha, None, op0=OP.mult)
        z0sq = singles.tile([128, TC], F32)
        nc.vector.tensor_mul(z0sq, z0, z0)
        nc.vector.tensor_scalar_mul(biasT_sb, z0sq, -0.5)

        # ---------- transpose s-columns -> lhsT rows ----------
        with tc.tile_pool(name="tr_ps", bufs=1, space="PSUM") as tr_ps:
            tr36 = tr_ps.tile([36, 128], BF16)
            eye_bf = singles.tile([128, 128], BF16)
            nc.vector.tensor_copy(eye_bf, eye_sb)
            nc.tensor.transpose(tr36, scol36, eye_bf)
            nc.vector.tensor_copy(lhs36_sb, tr36)

        if debug:
            nc.sync.dma_start(out=dbg["dbg_sig"][:, :], in_=sig4)
            nc.sync.dma_start(out=dbg["dbg_cols"][:, :], in_=cst)
            nc.sync.dma_start(out=dbg["dbg_lhs12"][:, :],
                              in_=lhs36_sb[0:12, :])
            nc.sync.dma_start(out=dbg["dbg_scale"][:, :], in_=scaleT_sb)
            nc.sync.dma_start(out=dbg["dbg_bias"][:, :], in_=biasT_sb)

        # helper: centered frac via magic-C round trick (x -> x - round(x))
        def frac_center(out_ap, in_ap, rtile, op0_scalar=None):
            """rtile: scratch F32 tile same shape; out = in - round(in)."""
            if op0_scalar is None:
                nc.vector.tensor_scalar(rtile, in_ap, C_MAGIC, C_MAGIC,
                                        op0=OP.add, op1=OP.subtract)
            else:
                # out = (in + s) - round(in + s): first materialize in+s
                raise NotImplementedError
            nc.vector.tensor_sub(out_ap, in_ap, rtile)

        # ---------- u_c tables: A64 = sin(2pi u_c), B64 = cos(2pi u_c) ----
        with tc.tile_pool(name="uc_ps", bufs=1, space="PSUM") as uc_ps, \
             tc.tile_pool(name="uc_sb", bufs=1) as uc_sb:
            ucp = uc_ps.tile([128, T], F32)
            nc.tensor.matmul(ucp, lhs36_sb[32:35, :], tramp2_sb[32:35, :],
                             start=True, stop=False, skip_group_check=True)
            nc.tensor.matmul(ucp, lhsCn, ones64_sb, start=False, stop=False,
                             skip_group_check=True)
            nc.tensor.matmul(ucp, lhsCp, ones64_sb, start=False, stop=False,
                             skip_group_check=True)
            nc.tensor.matmul(ucp, lhs36_sb[0:3, :], tramp2_sb[0:3, :],
                             start=False, stop=True, skip_group_check=True)
            # ucp = u_c in [-0.5, 0.5]
            nc.scalar.activation(A64_sb, ucp, A.Sin, bias=0.0, scale=TWO_PI)
            # cos via sin(2pi*(frac_center(u_c + 0.25)))
            uq = uc_sb.tile([128, T], F32, tag="uq")
            nc.vector.tensor_scalar_add(uq, ucp, 0.25)
            rq = uc_sb.tile([128, T], F32, tag="rq")
            uqf = uc_sb.tile([128, T], F32, tag="uqf")
            frac_center(uqf, uq, rq)
            nc.scalar.activation(B64_sb, uqf, A.Sin, bias=0.0, scale=TWO_PI)

        # banded rotation weights: Bbig[:, t, m] = B64[:, t] * delta(t == m)
        # (lhsT for the rotating event-sum matmuls; diagonal written with a
        # strided AP, rest zero)
        B64b = singles.tile([128, T], BF16)
        nc.vector.tensor_copy(B64b, B64_sb)
        A64b = singles.tile([128, T], BF16)
        nc.vector.tensor_copy(A64b, A64_sb)
        Bbig_sb = singles.tile([128, T, T], BF16)
        nc.vector.memset(Bbig_sb, 0.0)
        Abig_sb = singles.tile([128, T, T], BF16)
        nc.vector.memset(Abig_sb, 0.0)

        def diag_ap(big):
            base = big[:, 0, :]
            return bass.AP(tensor=base.tensor, offset=base.offset,
                           ap=[[base.ap[0][0], 128], [T + 1, T]])

        nc.vector.tensor_copy(diag_ap(Bbig_sb), B64b)
        nc.vector.tensor_copy(diag_ap(Abig_sb), A64b)

        # ---------- uM0 tables: sinM0 / cosM0 [128,512] fp16 ----------
        sinM0 = singles.tile([128, NT], FP16)
        cosM0 = singles.tile([128, NT], FP16)
        with tc.tile_pool(name="m0_ps", bufs=1, space="PSUM") as m0_ps, \
             tc.tile_pool(name="m0_sb", bufs=1) as m0_sb:
            m0p = m0_ps.tile([128, NT], F32)
            nc.tensor.matmul(m0p, lhs36_sb[0:6, :], basisA_sb,
                             start=True, stop=True)
            nc.vector.tensor_copy(M0_sb, m0p)
            um0 = m0_sb.tile([128, NT], F32, tag="um0")
            rm0 = m0_sb.tile([128, NT], F32, tag="rm0")
            frac_center(um0, M0_sb, rm0)
            nc.scalar.activation(sinM0, um0, A.Sin, bias=0.0, scale=TWO_PI)
            uq0 = m0_sb.tile([128, NT], F32, tag="uq0")
            nc.vector.tensor_scalar_add(uq0, um0, 0.25)
            rq0 = m0_sb.tile([128, NT], F32, tag="rq0")
            uqf0 = m0_sb.tile([128, NT], F32, tag="uqf0")
            frac_center(uqf0, uq0, rq0)
            nc.scalar.activation(cosM0, uqf0, A.Sin, bias=0.0, scale=TWO_PI)
            if debug:
                nc.sync.dma_start(out=dbg["dbg_um0"][:, :], in_=um0)
                nc.sync.dma_start(out=dbg["dbg_sinm0"][:, :], in_=sinM0)
                nc.sync.dma_start(out=dbg["dbg_cosm0"][:, :], in_=cosM0)
                nc.sync.dma_start(
                    out=dbg["dbg_AB"][:, 0:T], in_=A64_sb)
                nc.sync.dma_start(
                    out=dbg["dbg_AB"][:, T:2 * T], in_=B64_sb)

        # ---------- G' table and G'-premultiplied sin/cos tables ----------
        # (first Exp switches the ACT table set; no sins after this point)
        gp = singles.tile([128, CH], FP16)
        graw = singles.tile([128, CH], BF16)
        nc.scalar.activation(graw, j2_sb, A.Exp, bias=0.0, scale=na2h)
        nc.vector.tensor_scalar(gp, graw, cw, None, op0=OP.mult)
        for q in range(QP):
            nc.vector.tensor_mul(sinG_sb[:, ts(q, NT)], sinM0,
                                 gp[:, ts(q, NT)])
            nc.vector.tensor_mul(cosG_sb[:, ts(q, NT)], cosM0,
                                 gp[:, ts(q, NT)])
        if debug:
            nc.sync.dma_start(out=dbg["dbg_gp"][:, :], in_=gp)

        # ---------- fused hot loop ----------
        # out[t,:] = B[:,t]^T (E1*sinG)[:, jslice] + A[:,t]^T (E1*cosG)[...]
        # - the sine rotation rides inside the event-sum matmuls via the
        # delta-banded Bbig/Abig weights; the DVE only does two tensor-
        # tensor multiplies per 2048-chunk.
        with tc.tile_pool(name="hl_sb", bufs=2) as hl, \
             tc.tile_pool(name="po_ps", bufs=1, space="PSUM") as po_ps:
            out_ps = po_ps.tile([T, NT], F32)
            for c in range(TC):
                e1 = hl.tile([128, CH], FP16, tag="e1")
                nc.scalar.activation(e1, jramp_sb, A.Exp,
                                     bias=biasT_sb[:, c:c + 1],
                                     scale=scaleT_sb[:, c:c + 1])
                u1 = hl.tile([128, CH], BF16, tag="u1")
                nc.vector.tensor_mul(u1, e1, sinG_sb)
                u2 = hl.tile([128, CH], BF16, tag="u2")
                nc.vector.tensor_mul(u2, e1, cosG_sb)
                if debug and c == 0:
                    nc.sync.dma_start(out=dbg["dbg_e1"][:, :], in_=e1)
                    nc.sync.dma_start(out=dbg["dbg_sch"][:, :], in_=u1)
                    nc.sync.dma_start(out=dbg["dbg_p2"][:, :], in_=u2)
                for q in range(QP):
                    t = QP * c + q
                    nc.tensor.matmul(out_ps, Bbig_sb[:, t, :],
                                     u1[:, ts(q, NT)],
                                     start=(t == 0), stop=False,
                                     skip_group_check=True)
                    nc.tensor.matmul(out_ps, Abig_sb[:, t, :],
                                     u2[:, ts(q, NT)],
                                     start=False, stop=(t == T - 1),
                                     skip_group_check=True)

            # ---------- finalize: max-norm ----------
            if debug:
                outc = singles.tile([T, NT], F32)
                nc.vector.tensor_copy(outc, out_ps)
                nc.sync.dma_start(out=dbg["dbg_outps"][:, :], in_=outc)
            m1 = singles.tile([T, 1], F32)
            nc.vector.tensor_reduce(m1, out_ps, axis=mybir.AxisListType.X,
                                    op=OP.max, apply_absolute_value=True)
            with tc.tile_pool(name="fin_ps", bufs=1, space="PSUM") as fin_ps:
                m1t = fin_ps.tile([1, T], F32)
                nc.tensor.transpose(m1t, m1, eye_sb[0:T, 0:T])
                m1r = singles.tile([1, T], F32)
                nc.vector.tensor_copy(m1r, m1t)
                m2 = singles.tile([1, 1], F32)
                nc.vector.tensor_reduce(m2, m1r, axis=mybir.AxisListType.X,
                                        op=OP.max)
                nc.vector.tensor_scalar_add(m2, m2, 1e-12)
                inv = singles.tile([1, 1], F32)
                nc.vector.reciprocal(inv, m2)
                invb = fin_ps.tile([T, 1], F32)
                nc.tensor.matmul(invb, ones1_sb, inv, start=True, stop=True)
                inv64 = singles.tile([T, 1], F32)
                nc.vector.tensor_copy(inv64, invb)
                out_n = singles.tile([T, NT], F32)
                nc.vector.tensor_scalar(out_n, out_ps, inv64[:, :], None,
                                        op0=OP.mult)
                nc.sync.dma_start(out=out_d[:, :], in_=out_n)

    return nc


def _legalize_sync(nc):
    """Split multi-wait instructions into single-wait NOP prefixes."""
    from concourse import mybir

    n = 0
    for func in nc.m.functions:
        for block in func.blocks:
            out = []
            changed = False
            for inst in block.instructions:
                si = inst.sync_info
                if si is not None and len(si.on_wait) > 1:
                    waits = list(si.on_wait)
                    for w in waits[:-1]:
                        n += 1
                        nop = mybir.InstNoOp(name=f"lgl_wait_{n}")
                        nop.engine = inst.engine
                        nop.sync_info = mybir.SyncInfo(on_wait=[w],
                                                       on_update=[])
                        out.append(nop)
                    si.on_wait = [waits[-1]]
                    changed = True
                out.append(inst)
            if changed:
                block.instructions = out
    return n


def _get_nc():
    if "nc" not in _cached:
        nc = _build(debug=_cached.get("debug", False))
        _legalize_sync(nc)
        _cached["nc"] = nc
    return _cached["nc"]


def kernel(x, Ws, bs, Wo, bo):
    from concourse.bass_utils import run_bass_kernel_spmd

    x = np.asarray(x, np.float32).reshape(NB, E, D)
    Ws = np.asarray(Ws, np.float32)    # [4, 3, D, D]
    bs = np.asarray(bs, np.float32)    # [4, 3, D]
    Wo = np.asarray(Wo, np.float32)    # [4, 1, D]
    bo = np.asarray(bo, np.float32)    # [4, 1]

    # host-side layout prep (contiguous DMAs)
    ws_h = np.ascontiguousarray(
        Ws.reshape(12, D, D).transpose(2, 0, 1)).reshape(D, 12 * D)
    bs_h = np.ascontiguousarray(bs.reshape(12, D).T)        # [D, 12]
    wo_h = np.ascontiguousarray(Wo[:, 0, :].T)              # [D, 4]
    bo_h = np.ascontiguousarray(
        np.broadcast_to(bo[:, 0], (D, 4))).astype(np.float32)

    nc = _get_nc()
    in_maps = []
    for b in range(NB):
        in_maps.append({
            "xT": np.ascontiguousarray(x[b].T),
            "ws": ws_h, "bs": bs_h, "wo": wo_h, "bo": bo_h,
        })
    res = run_bass_kernel_spmd(nc, in_maps, core_ids=list(range(NB)),
                               **_cached.get("run_kwargs", {}))
    kernel.last_results = res
    outs = [res.results[b]["out"].reshape(1, N) for b in range(NB)]
    return np.stack(outs).astype(np.float32)
